# revision 40
# baseline (speedup 1.0000x reference)
"""Trainium2 Bass kernel for nn_Attention_57406532878693 (pooling attention).

Math (per (b, h) slice; T=2048, N=128, K2=16):
    x      = hyp[:, b, h*128:(h+1)*128]                    # (T, N)
    m      = x.mean(0)                                     # (N,)
    gx     = tanh(x @ W_w.T + W_b)                         # (T, K2)
    gm     = tanh(Wm_w @ m + Wm_b)                         # (K2,)
    u      = Wh_w[0] * gm                                  # (K2,)
    l      = gx @ u + Wh_b                                 # (T,)
    p      = exp(l)          (no max-sub needed: |l| <= 4.25, tanh-bounded)
    c      = (p @ x) / p.sum()                             # (N,)
    out[b, h*128:(h+1)*128] = c

Sharding: data-parallel over B across 8 cores (4 batches per core).

v4 design (vs the 368us baseline, which spent its time on 1288 PE
instructions — 512 transposes + 512 small per-chunk gate matmuls with
per-matmul stationary reloads + per-head bias/mean matmuls):

  - transposes stay on the PE (f32r, [128,128] via identity), but their
    PSUM evacuation now WRITES BF16 xt tiles laid out [n128, (i=4q+c, t)]
    so each head's 512 t-columns are contiguous.  (A DMA X-bar transpose
    variant was tried first: InstDmaTransposeAnt deterministically
    mangles even bf16 elements to sign|0x4000 whenever PE matmuls run
    concurrently, so it cannot be used in this kernel.)
  - gate matmuls run stationary-weight-style in bf16: gxT[k, t] with
    lhsT = [W_w.T | 0] (M=32, zero cols keep pad rows finite), rhs = xt
    512-col chunks, col-tiled 4 heads concurrent via
    tile_position=(0, 32q).  128 instrs instead of 512, no per-chunk
    stationary reloads, bias fused into the tanh via a per-partition
    bias column.
  - the time-mean path rides the same xt stream: a second accumulating
    bf16 matmul per chunk with lhsT = [Wm_w.T/T | 0] sums Wm@x in PSUM;
    one DVE free-reduce + ACT tanh + DVE mul produce u per head at
    partitions 32q+k, and a constant Wh_w block-mask turns that column
    into the block-diagonal U4 [128, 4] (bf16).
  - logits land t-major directly: lhsT = tanh(gxT) 128-col chunk (bf16,
    FWL-eligible), rhs = U4 -> l [t128, 4 heads]; exp + accum_out and
    the p_quad over-read weighted-sum (f32r, rhs = natural f32 tiles)
    are unchanged from the baseline.
  - KB_WARM=1 (default) sprinkles a tiny real matmul per transpose
    group: PE-mode transposes don't count as PE-busy for the HAM clock
    governor, and without the dummies the kernel is bistable between
    ~2.4GHz and ~1.2GHz PE clock (interleaved A/B: warm wins both call
    orders).

Measured via loop-slope bench (bench.py / bench_ab.py, tc.For_i at two
trip counts, paired-difference estimator; the axon RPC adds 60-110ms of
per-call noise and the device speed drifts run-to-run):
  baseline (prev session): 368018 ns/iter
  v4 (this config):        ~142-147 us/iter median, 94-115 us best
  TimelineSim structural estimate: 129 us/iter
PE instruction count per core-iteration: 1288 -> ~790 (512 transposes
kept, 512 gate matmuls -> 256 col-tiled bf16, bias/mean matmuls folded).
Numerics: rel err 1.86e-4 vs reference (bf16 gate path; f32r wsum).
"""

import os
import numpy as np

T, B, D = 2048, 32, 1024
H, N, K2 = 8, 128, 16
NCORES = 8
BL = B // NCORES          # 4 batches per core
TC = T // 128             # 16 t-chunks of 128
NQ = 4                    # nat tiles per (batch, head-quad); each holds 512 t
QW = 4 * N                # 512 cols per head-quad

LAST_RESULT = {}          # exec_time_ns etc. for test harness introspection


def _build(nc, tile, mybir, bass, whb_val, repeat=1, loop_n=0):
    f32 = mybir.dt.float32
    f32r = mybir.dt.float32r
    bf16 = mybir.dt.bfloat16
    AF = mybir.ActivationFunctionType

    hyp_s = nc.dram_tensor("hyp_s", [T, BL, D], f32, kind="ExternalInput").ap()
    ident_d = nc.dram_tensor("ident", [128, 128], f32, kind="ExternalInput").ap()
    wgz_d = nc.dram_tensor("wgz", [N, 32], bf16, kind="ExternalInput").ap()
    wmz_d = nc.dram_tensor("wmz", [N, 32], bf16, kind="ExternalInput").ap()
    wbc_d = nc.dram_tensor("wb_col", [128, 1], f32, kind="ExternalInput").ap()
    wmbc_d = nc.dram_tensor("wmb_col", [128, 1], f32, kind="ExternalInput").ap()
    whwm_d = nc.dram_tensor("whw_mask", [128, 4], f32, kind="ExternalInput").ap()
    ones_d = nc.dram_tensor("ones_col", [128, 2], f32, kind="ExternalInput").ap()
    out_s = nc.dram_tensor("out_s", [BL, D], f32, kind="ExternalOutput").ap()

    def r(ap):
        return ap.bitcast(f32r)

    with tile.TileContext(nc) as tc:
        from contextlib import ExitStack

        with ExitStack() as ctx:
            natf_b = int(os.environ.get("KB_NATF", "10"))
            xt_b = int(os.environ.get("KB_XT", "6"))
            g_b = int(os.environ.get("KB_G", "3"))
            pst_b = int(os.environ.get("KB_PST", "3"))
            psa_b = int(os.environ.get("KB_PSA", "2"))
            psb_b = int(os.environ.get("KB_PSB", "1"))
            psc_b = int(os.environ.get("KB_PSC", "1"))
            psw_b = int(os.environ.get("KB_PSW", "1"))
            cpool = ctx.enter_context(tc.tile_pool(name="consts", bufs=1))
            natf_pool = ctx.enter_context(tc.tile_pool(name="natf", bufs=natf_b))
            xt_pool = ctx.enter_context(tc.tile_pool(name="xt", bufs=xt_b))
            g_pool = ctx.enter_context(tc.tile_pool(name="g", bufs=g_b))
            sm_pool = ctx.enter_context(tc.tile_pool(name="small", bufs=6))
            out_pool = ctx.enter_context(tc.tile_pool(name="outp", bufs=1))
            pst_pool = ctx.enter_context(
                tc.tile_pool(name="pst", bufs=pst_b, space="PSUM"))
            psa_pool = ctx.enter_context(
                tc.tile_pool(name="psa", bufs=psa_b, space="PSUM"))
            psb_pool = ctx.enter_context(
                tc.tile_pool(name="psb", bufs=psb_b, space="PSUM"))
            psc_pool = ctx.enter_context(
                tc.tile_pool(name="psc", bufs=psc_b, space="PSUM"))
            psw_pool = ctx.enter_context(
                tc.tile_pool(name="psw", bufs=psw_b, space="PSUM"))

            ident = cpool.tile([128, 128], f32, tag="ident")
            nc.sync.dma_start(r(ident[:]), r(ident_d))
            wgz = cpool.tile([N, 32], bf16, tag="wgz")
            nc.sync.dma_start(wgz[:], wgz_d)
            wmz = cpool.tile([N, 32], bf16, tag="wmz")
            nc.sync.dma_start(wmz[:], wmz_d)
            wbc = cpool.tile([128, 1], f32, tag="wbc")
            nc.sync.dma_start(wbc[:], wbc_d)
            wmbc = cpool.tile([128, 1], f32, tag="wmbc")
            nc.sync.dma_start(wmbc[:], wmbc_d)
            whwm = cpool.tile([128, 4], f32, tag="whwm")
            nc.sync.dma_start(whwm[:], whwm_d)
            ones_c = cpool.tile([128, 2], f32, tag="ones")
            nc.sync.dma_start(r(ones_c[:]), r(ones_d))
            whb_c = cpool.tile([128, 1], f32, tag="whb")
            nc.gpsimd.memset(whb_c[:], float(whb_val))

            # PE transposes don't register as PE-busy for the HAM clock
            # governor, so a transpose-heavy stretch can drop the PE to
            # 1.2GHz.  KB_WARM sprinkles tiny real matmuls to keep the
            # activity window fed.  KB_PSD=1 gives them a dedicated PSUM
            # bank: when they share psC (bufs=1), the first dummy of each
            # quad carries a WAR wait on the previous quad's psC (freed
            # only after Z/recip), and the in-order PE queue then stalls
            # the next quad's transposes behind it.
            warm = os.environ.get("KB_WARM", "1") == "1"
            use_psd = os.environ.get("KB_PSD", "0") == "1"
            psd = None
            if warm and use_psd:
                psd_pool = ctx.enter_context(
                    tc.tile_pool(name="psd", bufs=1, space="PSUM"))
                psd = psd_pool.tile([1, 2], f32, tag="psd")

            def ham_warm(psC):
                if warm:
                    dst = psd[:] if psd is not None else psC[0:1, 66:68]
                    nc.tensor.matmul(dst, r(ones_c[:, 0:1]),
                                     r(ones_c[:, 0:2]),
                                     start=True, stop=True,
                                     skip_group_check=True)

            out_sb = out_pool.tile([97, BL * D // 4], f32, tag="out")

            def load_nat(b):
                tiles = []
                for hq in range(2):
                    row = []
                    for j in range(NQ):
                        t0 = j * 4 * 128
                        nt = natf_pool.tile([128, 4 * QW], f32, tag="natf")
                        src = hyp_s[t0:t0 + 4 * 128, b:b + 1,
                                    hq * QW:(hq + 1) * QW].rearrange(
                            "(c p) one d -> p c (one d)", p=128)
                        nc.sync.dma_start(
                            r(nt[:].rearrange("p (c d) -> p c d", c=4)),
                            r(src))
                        row.append(nt)
                    tiles.append(row)
                return tiles

            def do_batch(b, natf):
                for hq in range(2):
                    # logits/z/warm-dummy bank, allocated up front so the
                    # HAM-warmer matmuls can target its spare columns.
                    psC = psc_pool.tile([128, 68], f32, tag="psc")
                    # PE transposes -> PSUM -> bf16 evacuation into
                    # xt_j [n128, (i=4q+c, t)]: per-head t-cols contiguous.
                    xts = []
                    for j in range(NQ):
                        nf = natf[hq][j]
                        xt = xt_pool.tile([128, 4 * QW], bf16, tag="xt")
                        for q in range(4):
                            psT = pst_pool.tile([128, 512], f32, tag="pst")
                            ham_warm(psC)
                            for c in range(4):
                                nc.tensor.transpose(
                                    r(psT[:, 128 * c:128 * (c + 1)]),
                                    r(nf[:, 512 * c + 128 * q:
                                         512 * c + 128 * (q + 1)]),
                                    r(ident[:]))
                            dst = xt[:, 512 * q:512 * (q + 1)]
                            if (j + q) % 2 == 0:
                                nc.scalar.activation(dst, psT[:], AF.Copy)
                            else:
                                nc.vector.tensor_scalar(
                                    dst, psT[:], 1.0, 0.0,
                                    op0=mybir.AluOpType.mult,
                                    op1=mybir.AluOpType.add)
                        xts.append(xt)

                    g_sb = g_pool.tile([128, T], bf16, tag="g")
                    psB = psb_pool.tile([128, 512], f32, tag="psb")
                    for j in range(NQ):
                        psA = psa_pool.tile([128, 512], f32, tag="psa")
                        for q in range(4):
                            rhs = xts[j][:, 512 * q:512 * (q + 1)]
                            nc.tensor.matmul(
                                psA[32 * q:32 * q + 32, :], wgz[:], rhs,
                                start=True, stop=True,
                                tile_position=(0, 32 * q),
                                skip_group_check=True)
                        for q in range(4):
                            rhs = xts[j][:, 512 * q:512 * (q + 1)]
                            nc.tensor.matmul(
                                psB[32 * q:32 * q + 32, :], wmz[:], rhs,
                                start=(j == 0), stop=(j == NQ - 1),
                                tile_position=(0, 32 * q),
                                skip_group_check=True)
                        nc.scalar.activation(
                            g_sb[:, 512 * j:512 * (j + 1)], psA[:],
                            AF.Tanh, bias=wbc[:])

                    # mean-gate path: u at partitions 32q+k, col-masked to
                    # the block-diagonal U4 [128, 4] (bf16).
                    wmred = sm_pool.tile([128, 1], f32, tag="wmred")
                    nc.vector.tensor_reduce(
                        wmred[:], psB[:],
                        axis=mybir.AxisListType.X, op=mybir.AluOpType.add)
                    tanhc = sm_pool.tile([128, 1], f32, tag="tanhc")
                    nc.scalar.activation(tanhc[:], wmred[:], AF.Tanh,
                                         bias=wmbc[:])
                    u4 = sm_pool.tile([128, 4], bf16, tag="u4")
                    nc.vector.tensor_mul(
                        u4[:], whwm[:], tanhc[:].broadcast_to([128, 4]))

                    # logits t-major: l[t, q] = sum_p g[p, t] * U4[p, q]
                    for c in range(TC):
                        nc.tensor.matmul(
                            psC[:, 4 * c:4 * c + 4],
                            g_sb[:, 128 * c:128 * (c + 1)], u4[:],
                            start=True, stop=True, skip_group_check=True)

                    p_quad = sm_pool.tile([128, 144], f32, tag="p_quad")
                    pr_quad = sm_pool.tile([128, 97], f32, tag="pr_quad")
                    nc.gpsimd.memset(p_quad[:], 0.0)
                    nc.gpsimd.memset(pr_quad[:], 1.0)
                    lview = psC[:, 0:64].rearrange("p (c q) -> p q c", q=4)
                    with nc.allow_low_precision(
                            reason="f32r accum is fp32-width"):
                        for q in range(4):
                            nc.scalar.activation(
                                r(p_quad[:, 32 * q:32 * q + TC].unsqueeze(1)),
                                lview[:, q:q + 1, :],
                                AF.Exp, bias=whb_c[:],
                                accum_out=r(pr_quad[:, 32 * q:32 * q + 1]))

                    z_ps = psC[0:97, 64:66]
                    nc.tensor.matmul(z_ps, r(pr_quad[:]), r(ones_c[:]),
                                     start=True, stop=True,
                                     skip_group_check=True)
                    zi_sb = sm_pool.tile([97, 1], f32, tag="zi_sb")
                    nc.vector.reciprocal(zi_sb[:], z_ps[0:97, 0:1])

                    psW = psw_pool.tile([128, 512], f32, tag="psw")
                    for c in range(TC):
                        j, cl = c // 4, c % 4
                        rhs = natf[hq][j][:, 512 * cl:512 * (cl + 1)]
                        nc.tensor.matmul(psW[:], r(p_quad[:, c:c + 128]),
                                         r(rhs),
                                         start=(c == 0), stop=(c == TC - 1),
                                         skip_group_check=True)

                    for q in range(4):
                        col = b * (D // 4) + hq * N
                        nc.scalar.activation(
                            out_sb[32 * q:32 * q + 1, col:col + N],
                            psW[32 * q:32 * q + 1, q * N:(q + 1) * N],
                            AF.Copy, bias=0.0,
                            scale=zi_sb[32 * q:32 * q + 1, 0:1])

            if loop_n:
                with tc.For_i(0, loop_n, 1):
                    for b in range(BL):
                        do_batch(b, load_nat(b))
            else:
                sched = [bb for _ in range(repeat) for bb in range(BL)]
                for it, b in enumerate(sched):
                    do_batch(b, load_nat(b))

            for q in range(4):
                nc.sync.dma_start(
                    out_s.rearrange("b (j q n) -> q b j n", q=4, n=N)[q:q + 1],
                    out_sb[32 * q:32 * q + 1, :].rearrange(
                        "one (b j n) -> one b j n", j=H // 4, n=N))
    return nc


def _consts(inputs):
    import ml_dtypes
    W_w = np.asarray(inputs["W_w"], dtype=np.float32)      # (K2, N)
    W_b = np.asarray(inputs["W_b"], dtype=np.float32)      # (K2,)
    Wm_w = np.asarray(inputs["Wm_w"], dtype=np.float32)    # (K2, N)
    Wm_b = np.asarray(inputs["Wm_b"], dtype=np.float32)    # (K2,)
    Wh_w = np.asarray(inputs["Wh_w"], dtype=np.float32)    # (1, K2)

    bf = ml_dtypes.bfloat16
    wgz = np.zeros((N, 32), np.float32)
    wgz[:, 0:K2] = W_w.T
    wmz = np.zeros((N, 32), np.float32)
    wmz[:, 0:K2] = Wm_w.T / T
    wbc = np.zeros((128, 1), np.float32)
    wmbc = np.zeros((128, 1), np.float32)
    whwm = np.zeros((128, 4), np.float32)
    for q in range(4):
        wbc[32 * q:32 * q + K2, 0] = W_b
        wmbc[32 * q:32 * q + K2, 0] = Wm_b
        whwm[32 * q:32 * q + K2, q] = Wh_w[0]
    return {
        "ident": np.eye(128, dtype=np.float32),
        "wgz": wgz.astype(bf),
        "wmz": wmz.astype(bf),
        "wb_col": wbc,
        "wmb_col": wmbc,
        "whw_mask": whwm,
        "ones_col": np.ones((128, 2), np.float32),
    }


def kernel(**inputs):
    import concourse.bass as bass
    import concourse.bacc as bacc
    import concourse.tile as tile
    import concourse.mybir as mybir
    from concourse import bass_utils

    hyp = np.ascontiguousarray(np.asarray(inputs["hyp"], dtype=np.float32))
    Wh_b = np.asarray(inputs["Wh_b"], dtype=np.float32)    # (1,)

    nc = bacc.Bacc("TRN2", target_bir_lowering=False, debug=False)
    _build(nc, tile, mybir, bass, float(Wh_b.reshape(-1)[0]))
    nc.compile()

    consts = _consts(inputs)
    in_maps = []
    for j in range(NCORES):
        m = {"hyp_s": np.ascontiguousarray(hyp[:, j * BL:(j + 1) * BL, :])}
        m.update(consts)
        in_maps.append(m)

    trace = os.environ.get("BASS_KERNEL_TRACE", "0") == "1"
    res = bass_utils.run_bass_kernel_spmd(
        nc, in_maps, core_ids=list(range(NCORES)), trace=trace)

    LAST_RESULT.clear()
    LAST_RESULT["exec_time_ns"] = res.exec_time_ns
    LAST_RESULT["trace"] = (res.instructions_and_trace[1]
                            if res.instructions_and_trace else None)
    LAST_RESULT["profile_json"] = res.profile_json

    out = np.concatenate([res.results[j]["out_s"] for j in range(NCORES)],
                         axis=0)
    return out.astype(np.float32)
